# revision 49
# baseline (speedup 1.0000x reference)
"""Trainium2 Bass kernel for nn_Attention_36799279792519.

Full causal self-attention layer (QKV proj + RoPE + causal softmax attention +
output proj), B=2 T=2048 C=1024 H=16 D=64, sharded over 8 NeuronCores:
data-parallel on batch (2) x tensor-parallel on heads (4 heads/core).
Each core computes its heads' attention output and a partial projection
(T, C) in f32; the host sums the 4 partials per batch and adds proj bias.

v5 design (vs the v2 baseline, 129.6us -> 123.0us cost-model time):
  * scores via fp8e4m3 DoubleRow matmuls at 0.5 cyc/row (half the bf16
    PE time) with ASYMMETRIC hi/lo: k is stored as an (hi, lo) fp8 pair
    in the DoubleRow sub-slots, q as a single fp8 broadcast into both
    sub-slots via a stride-0 moving AP, so each score block is ONE DR
    matmul computing (kh+kl)^T qh - only the q side carries fp8
    quantization error (~1.4% on the output, inside the 2e-2 budget).
  * causal masks: gpsimd (Pool) affine_select zeroes the upper triangle
    of each diagonal at-block after the exp (Pool is otherwise idle),
    removing 64 mask matmuls from the PE stream.
  * merged transposes: one [128,128] identity matmul transposes both
    heads' normalized [t,d] sub-chunk at once (halves transpose count
    and PE rows vs per-head transposes).
  * round 0 scores in bf16 straight from the rope (ab) layout - no
    chunk-0 q permute, shorter prologue, and exact round-0 logits.
  * q-outer attention: for each 512-wide q chunk j, iterate key blocks kb,
    per-head score psum tiles; exp on Act -> bf16 at tiles.  Paired-step
    emission (2 blocks per step, scores 4-5 iterations ahead, pv lagging
    4) keeps the Act exp stream ~gapless in steady state; the last
    round-pair emits its diagonal blocks first so the tail pv/normalize/
    transpose/proj chains pipeline behind the last full-block exp.
  * input DMAs merged (x hi+lo one tensor, qkv weights one tensor,
    cos/sin and biases packed) and the late x/wp chunks emitted as fills
    so the serial DMA issue path (~650ns per DMA) doesn't starve the
    early rounds' permute DMAs.  The four chunk-0 QKV groups get four
    distinct psum homes (sc/sc/fl/pv) so the prologue doesn't serialize
    on one psum rotation; chunk-0's k rope and deferred k permconv run
    in 256-col halves; fill-chunk qkv biases apply on Act while it is
    idle in rounds 0-1.
  * PV transposed (bf16): out[t,d] = at[k,t]^T @ v[k,d] -> [128t, 65]
    matmuls; the ones-column of V accumulates the softmax denominator per
    partition, so normalization is a per-partition reciprocal + mul.
  * normalized [t,d] tiles are PE-transposed back to [d,t] via an identity
    matmul into a bf16 psum tile, then copied to SBUF for the projection.
  * projection accumulates [128,512] f32 psum halves, evacuated to bf16
    SBUF staging and DMA'd per 128-row chunk.
  * everything pipelined: per-512-col QKV chunks + per-chunk rope;
    attention rounds j=0..3; k/q/v/proj groups for later rounds are
    sprinkled into the attention instruction stream as PE fillers.

Self-contained: hardcodes all shapes; no sibling imports.
"""
import numpy as np
import ml_dtypes

import concourse.bass as bass
import concourse.mybir as mybir
import concourse.tile as tile
from concourse import bacc
from concourse.bass_utils import run_bass_kernel_spmd

B, T, C = 2, 2048, 1024
H, D = 16, 64
SCALE = D ** -0.5
NCORES = 8
CORES_PER_B = NCORES // B          # 4
HPC = H // CORES_PER_B             # 4 heads per core
RL = HPC * D                       # 256 local q/k/v rows
CCH = C // 128                     # 8 contraction chunks
NJ = T // 512                      # 4 q chunks of 512
KB = T // 128                      # 16 key blocks of 128

F32 = mybir.dt.float32
BF16 = mybir.dt.bfloat16
F8 = mybir.dt.float8e4
BF = ml_dtypes.bfloat16

_compiled = {}


def _build():
    nc = bacc.Bacc("TRN2", target_bir_lowering=False, debug=False,
                   num_devices=NCORES)

    d = {}
    d["x8"] = nc.dram_tensor("x8", [2 * C, T], F8, kind="ExternalInput").ap()
    d["wqkv"] = nc.dram_tensor("wqkv", [C, 3 * 2 * RL], F8,
                               kind="ExternalInput").ap()
    d["wp"] = nc.dram_tensor("wproj_t", [RL, C], BF16, kind="ExternalInput").ap()
    d["bqk"] = nc.dram_tensor("bqk", [128, 4], F32, kind="ExternalInput").ap()
    d["bv"] = nc.dram_tensor("bv", [RL], F32, kind="ExternalInput").ap()
    d["cs"] = nc.dram_tensor("cs", [2 * 128, T], BF16, kind="ExternalInput").ap()
    d["ident"] = nc.dram_tensor("ident", [128, 128], BF16,
                                kind="ExternalInput").ap()
    d["out"] = nc.dram_tensor("out", [T, C], BF16, kind="ExternalOutput").ap()

    with tile.TileContext(nc) as tc:
        _program(nc, tc, d)

    nc.compile()
    return nc


def _program(nc, tc, d):
    AF = mybir.ActivationFunctionType
    with (
        tc.tile_pool(name="const", bufs=1) as const,
        tc.tile_pool(name="qk", bufs=1) as qkpool,
        tc.tile_pool(name="work", bufs=2) as work,
        tc.tile_pool(name="ps_sc", bufs=2, space="PSUM") as ps_sc,
        tc.tile_pool(name="ps_fl", bufs=1, space="PSUM") as ps_fl,
        tc.tile_pool(name="ps_pv", bufs=1, space="PSUM") as ps_pv,
        tc.tile_pool(name="ps_tr", bufs=1, space="PSUM") as ps_tr,
    ):
        # ================= long-lived SBUF tiles =================
        # hi+lo fp8 split of x and the qkv weights (weights pre-scaled x32
        # on the host): x @ w ~= (xh+xl) @ (wh+wl) dropping the lo*lo term.
        # Layout [128, ch, tl, .]: contraction row = ch*256 + tl*128 + p,
        # ready for DoubleRow matmuls (256-row reduction tiles).
        x_sb = const.tile([128, 2, 4, 2, T], F8)   # (hl, ch, tl, t)
        xh_sb = x_sb[:, 0]
        xl_sb = x_sb[:, 1]
        w_sb = const.tile([128, 4, 2, 3, 2 * RL], F8, name="wqkv")
        wq_sb, wk_sb, wv_sb = (0, 1, 2)  # selector index into w_sb dim 3
        wp_sb = const.tile([128, 2, C], BF16)
        cs_sb = const.tile([128, 2, T], BF16)
        ck_sb = cs_sb[:, 0]
        sk_sb = cs_sb[:, 1]
        ident_sb = const.tile([128, 128], BF16)
        bqk_sb = const.tile([128, 4], F32)
        bq_sb = bqk_sb[:, 0:2]
        bk_sb = bqk_sb[:, 2:4]
        bv_bc = const.tile([128, RL], F32)


        # rotated q/k, ab layout: [h*32+i, s, t] = rotated dim s*32+i of
        # head h (s=0: a*cos-b*sin, s=1: a*sin+b*cos)
        qrd = qkpool.tile([128, 2, T], BF16, tag="qrd", name="qrd")
        krd = qkpool.tile([128, 2, T], BF16, tag="krd", name="krd")
        # permuted + fp8-converted copies for the score matmuls:
        # khl[pair][p, hl, t]: rows hh*64+d head-contiguous, k hi/lo pair;
        # qh8[pair][p, t]: same rows, single fp8 (dup'd via stride-0 AP)
        khl = [qkpool.tile([128, 2, T], F8, tag=f"khl{i}", name=f"khl{i}")
               for i in range(2)]
        qh8 = [qkpool.tile([128, T], F8, tag=f"qh8{i}", name=f"qh8{i}")
               for i in range(2)]
        v_sb = qkpool.tile([128, KB, HPC, 65], BF16, tag="v")
        oT = [qkpool.tile([128, T], BF16, tag=f"oT{i}", name=f"oT{i}")
              for i in range(2)]

        nc.vector.memset(v_sb[:, :, :, 64:65], 1.0)

        # ================= input DMAs (ordered for the pipeline) ========
        x_r = d["x8"].rearrange("(hl ch tl p) t -> p hl ch tl t",
                                p=128, tl=2, ch=4)
        w_r = d["wqkv"].rearrange("(ch tl p) (w r) -> p ch tl w r",
                                  p=128, tl=2, w=3)
        cs_r = d["cs"].rearrange("(s p) t -> p s t", p=128)
        nc.sync.dma_start(out=ident_sb, in_=d["ident"])
        nc.sync.dma_start(out=w_sb[:, :, :, 0, :], in_=w_r[:, :, :, 0, :])
        nc.sync.dma_start(out=x_sb[:, 0, :, :, 0:512],
                          in_=x_r[:, 0, :, :, 0:512])
        nc.sync.dma_start(out=w_sb[:, :, :, 1, :], in_=w_r[:, :, :, 1, :])
        nc.sync.dma_start(out=x_sb[:, 1, :, :, 0:512],
                          in_=x_r[:, 1, :, :, 0:512])
        nc.sync.dma_start(out=cs_sb[:, :, 0:512], in_=cs_r[:, :, 0:512])
        nc.sync.dma_start(out=w_sb[:, :, :, 2, :], in_=w_r[:, :, :, 2, :])
        nc.sync.dma_start(out=bqk_sb, in_=d["bqk"])
        nc.sync.dma_start(
            out=bv_bc,
            in_=bass.AP(tensor=d["bv"].tensor, offset=d["bv"].offset,
                        ap=[[0, 128]] + list(d["bv"].ap)))
        nc.sync.dma_start(out=x_sb[:, :, :, :, 512:1024],
                          in_=x_r[:, :, :, :, 512:1024])

        # x chunks 2-3, the cos/sin tails, and the proj weights are not
        # needed until rounds 1-2; deferring their emission drops their
        # scheduler priority below the chunk-0/1 permute DMAs so the
        # (serial) DMA pipe doesn't starve the first attention rounds.
        def dma_fills():
            out = [lambda: nc.sync.dma_start(out=cs_sb[:, :, 512:2048],
                                             in_=cs_r[:, :, 512:2048])]
            for lo, hi in ((1024, 1536), (1536, 2048)):
                out.append(lambda lo=lo, hi=hi: nc.sync.dma_start(
                    out=x_sb[:, :, :, :, lo:hi], in_=x_r[:, :, :, :, lo:hi]))
            out.append(lambda: nc.sync.dma_start(
                out=wp_sb, in_=d["wp"].rearrange("(dc p) c -> p dc c", p=128)))
            return out

        # pull the Exp table load out of the critical path
        warm = work.tile([128, 1], F32, tag="warm", name="warm", bufs=1)
        nc.vector.memset(warm, 0.0)
        nc.scalar.activation(out=warm, in_=warm, func=AF.Exp)

        # PE warm-up: the Tensor engine p-state ramps to full clock only
        # after ~3us of continuous execution, and the first QKV matmuls
        # can't start until the x/w DMAs land (~4.4us).  Chew on the ident
        # tile meanwhile so the array is at full speed when real work
        # arrives (also keeps pe_busy_start from resetting).
        wu_ps = ps_tr.tile([128, 512], BF16, tag="tr", name="warmup")
        for _ in range(34):
            nc.tensor.transpose(wu_ps[:, 0:128], ident_sb, ident_sb)

        # ================= emission helpers =================
        DR = mybir.MatmulPerfMode.DoubleRow
        INV32 = 1.0 / 32.0

        def qkv_group(wsel, b_sb, dst_ab, rc, j, on_act, pro=False):
            lo, hi = j * 512, (j + 1) * 512
            # prologue groups each get their own psum home (sc/sc/fl/pv)
            # so the four chunk-0 groups don't serialize on one rotation
            if pro == "sc":
                ps = ps_sc.tile([128, 2, 512], F32, tag="sc",
                                name=f"qkv{rc}{j}")[:, 0, :]
            elif pro == "pv":
                ps = ps_pv.tile([128, 2, 512], F32, tag="pv",
                                name=f"qkv{rc}{j}")[:, 0, :]
            else:
                ps = ps_fl.tile([128, 512], F32, tag="fl",
                                name=f"qkv{rc}{j}")
            k = 0
            for wo, b in ((0, xh_sb), (RL, xh_sb), (0, xl_sb)):
                for ch in range(4):
                    nc.tensor.matmul(
                        ps,
                        w_sb[:, ch, :, wsel,
                             wo + rc * 128:wo + (rc + 1) * 128],
                        b[:, ch, :, lo:hi],
                        start=(k == 0), stop=(k == 11), perf_mode=DR)
                    k += 1
            if on_act:
                nc.scalar.activation(out=dst_ab[rc], in_=ps,
                                     func=AF.Identity, scale=INV32,
                                     bias=b_sb[:, rc:rc + 1])
            else:
                nc.vector.tensor_scalar(
                    dst_ab[rc], ps, INV32, b_sb[:, rc:rc + 1],
                    op0=mybir.AluOpType.mult, op1=mybir.AluOpType.add)

        def v_group(kb):
            ps = ps_fl.tile([128, 512], F32, tag="fl", name=f"v{kb}")
            psv = ps[:, 0:RL]
            k = 0
            for a, wo in ((xh_sb, 0), (xh_sb, RL), (xl_sb, 0)):
                for ch in range(4):
                    nc.tensor.matmul(
                        psv, a[:, ch, :, kb * 128:(kb + 1) * 128],
                        w_sb[:, ch, :, wv_sb, wo:wo + RL],
                        start=(k == 0), stop=(k == 11), perf_mode=DR)
                    k += 1
            nc.vector.scalar_tensor_tensor(
                out=v_sb[:, kb, :, 0:64],
                in0=psv.rearrange("p (h dd) -> p h dd", h=HPC),
                scalar=INV32,
                in1=bv_bc.rearrange("p (h dd) -> p h dd", h=HPC),
                op0=mybir.AluOpType.mult,
                op1=mybir.AluOpType.add)

        def rope_chunk(ab, rd, j, clo=0, chi=512):
            lo, hi = j * 512 + clo, j * 512 + chi
            w = chi - clo
            t1 = work.tile([128, 512], BF16, tag="rt1", name="rt1")[:, 0:w]
            t2 = work.tile([128, 512], BF16, tag="rt2", name="rt2")[:, 0:w]
            nc.vector.tensor_mul(t1, ab[0][:, clo:chi], ck_sb[:, lo:hi])
            nc.vector.tensor_mul(t2, ab[1][:, clo:chi], sk_sb[:, lo:hi])
            nc.vector.tensor_sub(rd[:, 0, lo:hi], t1, t2)
            t3 = work.tile([128, 512], BF16, tag="rt1", name="rt3")[:, 0:w]
            t4 = work.tile([128, 512], BF16, tag="rt2", name="rt4")[:, 0:w]
            nc.vector.tensor_mul(t3, ab[0][:, clo:chi], sk_sb[:, lo:hi])
            nc.vector.tensor_mul(t4, ab[1][:, clo:chi], ck_sb[:, lo:hi])
            nc.vector.tensor_add(rd[:, 1, lo:hi], t3, t4)

        def permconv_chunk(rd, j, is_k, clo=0, chi=512):
            # one DMA per pair: [64, 2, w] -> [128, w] in flat run order,
            # leaving each head's 64 dims INTERLEAVED (d0t, d0b, d1t, ...).
            # Scores only contract over these rows, and q and k share the
            # same order, so the interleave is harmless.  Then convert to
            # fp8: k gets an (hi, lo) pair, q a single hi.
            lo, hi = j * 512 + clo, j * 512 + chi
            w = chi - clo
            for pair in range(2):
                pb = work.tile([128, 512], BF16, tag=f"pm{is_k}{pair}",
                               name=f"pm{is_k}{pair}{j}{clo}", bufs=2)[:, 0:w]
                nc.sync.dma_start(
                    out=pb, in_=rd[pair * 64:(pair + 1) * 64, :, lo:hi])
                if is_k:
                    # hi convert on the (otherwise idle) Pool engine; the
                    # lo residual needs mixed-dtype subtract -> DVE
                    nc.gpsimd.tensor_copy(khl[pair][:, 0, lo:hi], pb)
                    nc.vector.tensor_sub(khl[pair][:, 1, lo:hi], pb,
                                         khl[pair][:, 0, lo:hi])
                else:
                    nc.gpsimd.tensor_copy(qh8[pair][:, lo:hi], pb)

        def k_chunk(j, on_act=False, pro=False):
            k_ab = [work.tile([128, 512], BF16, tag=f"kab{i}",
                              name=f"kab{i}{j}") for i in range(2)]
            qkv_group(wk_sb, bk_sb, k_ab, 0, j, on_act, "sc" if pro else False)
            qkv_group(wk_sb, bk_sb, k_ab, 1, j, on_act and not pro,
                      "sc" if pro else False)
            if pro:
                # split the prologue k rope so scores kb=0,1 can start
                # right after the first half lands
                rope_chunk(k_ab, krd, j, 0, 256)
                rope_chunk(k_ab, krd, j, 256, 512)
            else:
                rope_chunk(k_ab, krd, j)

        def q_chunk(j, on_act=False, pro=False):
            q_ab = [work.tile([128, 512], BF16, tag=f"qab{i}",
                              name=f"qab{i}{j}") for i in range(2)]
            qkv_group(wq_sb, bq_sb, q_ab, 0, j, on_act, "fl" if pro else False)
            qkv_group(wq_sb, bq_sb, q_ab, 1, j, on_act and not pro,
                      "pv" if pro else False)
            rope_chunk(q_ab, qrd, j)

        def proj_half(t16, half, tail=False):
            if tail:  # score psum pool is free after the last exp
                ps = ps_sc.tile([128, 2, 512], F32, tag="sc",
                                name=f"pj{t16}{half}")[:, 0, :]
            else:
                ps = ps_fl.tile([128, 512], F32, tag="fl",
                                name=f"pj{t16}{half}")
            for dc in range(2):
                nc.tensor.matmul(
                    ps,
                    oT[dc][:, t16 * 128:(t16 + 1) * 128],
                    wp_sb[:, dc, half * 512:(half + 1) * 512],
                    start=(dc == 0), stop=(dc == 1))
            o_sb = osb_tiles[t16 % 3]
            if tail and half == 0:
                nc.scalar.copy(o_sb[:, half * 512:(half + 1) * 512], ps)
            else:
                nc.vector.tensor_copy(o_sb[:, half * 512:(half + 1) * 512],
                                      ps)
            if half == 1:
                nc.sync.dma_start(
                    out=d["out"][t16 * 128:(t16 + 1) * 128, :], in_=o_sb)

        osb_tiles = [qkpool.tile([128, C], BF16, tag=f"osb{i}",
                                 name=f"osb{i}") for i in range(3)]

        # ============ software-pipelined attention + fills ============
        # last round-pair emits its pure-diagonal blocks' scores FIRST so
        # the tail's pv->normalize->transpose->proj chains pipeline right
        # after the last full-block exp instead of serializing at the end
        iters = []
        for j in range(NJ):
            for pair in range(2):
                if (pair, j) == (1, NJ - 1):
                    order = [13, 14, 15] + list(range(13))
                else:
                    order = list(range(4 * j + 4))
                for kb in order:
                    iters.append((pair, j, kb))

        state = {}  # (pair, j) -> dict(pv, at map, rec, o_n)

        def emit_scores(it):
            pair, j, kb = it
            q0 = 512 * j
            k0 = kb * 128
            qlo = max(q0, k0)
            w = 512 - (qlo - q0)
            st = state.setdefault((pair, j), {})
            if "pv" not in st:
                st["pv"] = ps_pv.tile([128, 2, 512], F32, tag="pv",
                                      name=f"pv{pair}{j}")
                st["rec"] = work.tile([128, 2, 4, 1], F32, tag="rec",
                                      name=f"rec{pair}{j}")
                st["o_n"] = work.tile([128, 4, 2, 64], BF16, tag="on",
                                      name=f"on{pair}{j}")
            ps = ps_sc.tile([128, 2, 512], F32, tag="sc",
                            name=f"sc{pair}{j}{kb}")
            diag = kb >= 4 * j
            for hh in range(2):
                if j == 0:
                    # chunk-0 q/k not yet permuted: contract the two rope
                    # halves separately from the ab-layout tiles (bf16)
                    h = 2 * pair + hh
                    for s in range(2):
                        nc.tensor.matmul(
                            ps[:, hh, 0:w],
                            krd[h * 32:(h + 1) * 32, s, k0:k0 + 128],
                            qrd[h * 32:(h + 1) * 32, s, qlo:q0 + 512],
                            start=(s == 0), stop=(s == 1),
                            tile_position=(h * 32, 0))
                    continue
                qs = qh8[pair][hh * 64:(hh + 1) * 64, qlo:q0 + 512]
                qdup = bass.AP(tensor=qs.tensor, offset=qs.offset,
                               ap=[list(qs.ap[0]), [0, 2]]
                               + [list(p) for p in qs.ap[1:]])
                nc.tensor.matmul(
                    ps[:, hh, 0:w],
                    khl[pair][hh * 64:(hh + 1) * 64, :, k0:k0 + 128],
                    qdup,
                    start=True, stop=True, perf_mode=DR,
                    tile_position=(hh * 64, 0))
            at = work.tile([128, 2, 512], BF16, tag=f"at{kb}",
                           name=f"at{pair}{j}{kb}",
                           bufs=(3 if kb < 6 else 2))
            nc.scalar.activation(out=at[:, :, 0:w], in_=ps[:, :, 0:w],
                                 func=AF.Exp, scale=float(SCALE))
            if diag:  # zero the upper triangle of the diagonal block (Pool)
                nc.gpsimd.affine_select(
                    out=at[:, :, 0:128], in_=at[:, :, 0:128],
                    pattern=[[0, 2], [1, 128]],
                    compare_op=mybir.AluOpType.is_ge,
                    fill=0.0, base=0, channel_multiplier=-1)
            st[kb] = at

        def emit_pv(it):
            # PSUM zero regions (banks) allow only ONE live accumulation
            # group: per head-bank, the tq sub-chunks accumulate
            # SEQUENTIALLY into pv[:, hh, 0:65].  tq0 streams along with
            # the exps (in kb order); the tq=m>=1 replay bursts fire once
            # every block they read has arrived (kb emission order may put
            # the diagonal blocks first, see the last-round reorder).
            pair, j, kb = it
            q0 = 512 * j
            st = state[(pair, j)]
            pv = st["pv"]
            arr = st.setdefault("arrived", set())
            arr.add(kb)
            if kb <= 4 * j and all(k2 in arr for k2 in range(kb)):
                # tq0 streaming group, in-order: emit every streamable
                # block that has arrived (handles out-of-order arrivals)
                nxt = st.setdefault("streamed", 0)
                while nxt <= 4 * j and nxt in arr:
                    at = st[nxt]
                    for hh in range(2):
                        nc.tensor.matmul(
                            pv[:, hh, 0:65], at[:, hh, 0:128],
                            v_sb[:, nxt, 2 * pair + hh, :],
                            start=(nxt == 0), stop=(nxt == 4 * j))
                    nxt += 1
                st["streamed"] = nxt
            mdone = st.setdefault("mdone", 0)
            while mdone < 4:
                m = mdone
                if m == 0:
                    ready = st.get("streamed", 0) > 4 * j
                else:
                    ready = all(k2 in arr for k2 in range(4 * j + m + 1))
                if not ready:
                    break
                if m >= 1:  # replay sweep for sub-chunk tq = m
                    tqq = q0 + 128 * m
                    for hh in range(2):
                        h = 2 * pair + hh
                        for kb2 in range(4 * j + m + 1):
                            qlo2 = max(q0, kb2 * 128)
                            nc.tensor.matmul(
                                pv[:, hh, 0:65],
                                st[kb2][:, hh, tqq - qlo2:tqq - qlo2 + 128],
                                v_sb[:, kb2, h, :],
                                start=(kb2 == 0), stop=(kb2 == 4 * j + m))
                # sub-chunk tq = m is complete: normalize it now
                for hh in range(2):
                    nc.vector.reciprocal(st["rec"][:, hh, m],
                                         pv[:, hh, 64:65])
                    nc.vector.tensor_scalar_mul(
                        st["o_n"][:, m, hh, :], pv[:, hh, 0:64],
                        st["rec"][:, hh, m, :])
                if m == 0:
                    st["tr"] = ps_tr.tile([128, 512], BF16, tag="tr",
                                          name=f"tr{pair}{j}")
                pend_tr.append((pair, j, m))
                mdone += 1
                st["mdone"] = mdone
                if m == 3:
                    for kb2 in range(4 * j + 4):
                        st.pop(kb2)

        def emit_transposes(pair, j, tq):
            # one [128,128] transpose covers both heads' [t, d] sub-chunks:
            # in [128t, (hh,d)] -> out [(hh,d), t], matching the oT layout
            st = state[(pair, j)]
            nc.tensor.transpose(
                st["tr"][:, tq * 128:(tq + 1) * 128],
                st["o_n"][:, tq, :, :], ident_sb)
            nc.vector.tensor_copy(
                oT[pair][:, 512 * j + 128 * tq:512 * j + 128 * (tq + 1)],
                st["tr"][:, tq * 128:(tq + 1) * 128])
            if pair == 1 and j == NJ - 1:
                # last round: project this 128-row chunk immediately
                proj_half(4 * j + tq, 0, tail=True)
                proj_half(4 * j + tq, 1, tail=True)

        # ================= schedule =================
        # prologue: chunk-0 q then k (q rope fully gates the first score
        # window; k only needs its first cols) + first v blocks
        q_chunk(0, on_act=True, pro=True)
        k_chunk(0, on_act=True, pro=True)

        round_fills = {j: [] for j in range(NJ)}

        def k_fills(jj):
            def k_all():
                # chunks 1-2 apply the qkv bias on Act (idle in rounds
                # 0-1) to keep the DVE-serialized chunk chain short;
                # chunk 3 stays on DVE (Act is the pacer by then)
                k_chunk(jj, on_act=(jj <= 2))
                permconv_chunk(krd, jj, True)
            return [k_all]

        def q_fills(jj):
            def q_all():
                q_chunk(jj, on_act=(jj <= 2))
                permconv_chunk(qrd, jj, False)
            return [q_all]

        def v_fills(kbs):
            return [lambda kk=kb: v_group(kk) for kb in kbs]

        def p_fills(t16s):
            out = []
            for t16 in t16s:
                out.append(lambda tt=t16: proj_half(tt, 0))
                out.append(lambda tt=t16: proj_half(tt, 1))
            return out

        # placement balances per-round PE fill work against the growing
        # exp (Act) load; chunk k(j)/q(j) must precede round j's scores
        dmaf = dma_fills()
        round_fills[0] = ([lambda: permconv_chunk(krd, 0, True, 0, 256),
                           lambda: permconv_chunk(krd, 0, True, 256, 512),
                           dmaf[0]]
                          + q_fills(1) + k_fills(1)
                          + [dmaf[1]] + v_fills(range(4, 8)))
        round_fills[1] = (q_fills(2) + k_fills(2) + [dmaf[2], dmaf[3]]
                          + p_fills(range(2)))
        round_fills[2] = (v_fills(range(8, 12)) + q_fills(3) + k_fills(3)
                          + p_fills(range(2, 8)))
        round_fills[3] = v_fills(range(12, 16)) + p_fills(range(8, 12))

        def pop_fill(cur_j):
            for jj in range(cur_j + 1):  # earliest unfinished fills first
                if round_fills[jj]:
                    round_fills[jj].pop(0)()
                    return

        # paired-step emission: scores+exp for TWO blocks are emitted a full
        # step ahead of their PV consumers, so the Act exp stream stays
        # gapless while PE absorbs the cross-engine semaphore latency once
        # per pair instead of once per block.  Transposes for sub-chunk tq
        # lag behind their normalize.
        pend_tr = []
        pend_pv = []
        emit_scores(iters[0])
        emit_scores(iters[1])
        v_group(0)
        v_group(1)
        emit_scores(iters[2])
        emit_scores(iters[3])
        v_group(2)
        v_group(3)
        for s in range(0, len(iters), 2):
            if s + 4 < len(iters):
                emit_scores(iters[s + 4])
            if s + 5 < len(iters):
                emit_scores(iters[s + 5])
            pend_pv.append(iters[s])
            pend_pv.append(iters[s + 1])
            # pv lags one step behind its exp so the PE queue head never
            # waits on the Act->Pool chain of the block it consumes
            pop_fill(iters[s][1])
            if len(pend_pv) > 4:
                if pend_tr:
                    emit_transposes(*pend_tr.pop(0))
                emit_pv(pend_pv.pop(0))
            pop_fill(iters[s + 1][1])
            if len(pend_pv) > 4:
                if pend_tr:
                    emit_transposes(*pend_tr.pop(0))
                emit_pv(pend_pv.pop(0))
        while pend_pv:
            if pend_tr:
                emit_transposes(*pend_tr.pop(0))
            emit_pv(pend_pv.pop(0))
            if pend_tr:
                emit_transposes(*pend_tr.pop(0))
        while pend_tr:
            emit_transposes(*pend_tr.pop(0))
        for j in range(NJ):
            while round_fills[j]:
                round_fills[j].pop(0)()


F8NP = ml_dtypes.float8_e4m3fn


def _split8(a):
    """bf16-valued f32 array -> (hi, lo) fp8 pair with a ~= hi + lo."""
    hi = a.astype(F8NP)
    lo = (a - hi.astype(np.float32)).astype(F8NP)
    return np.ascontiguousarray(hi), np.ascontiguousarray(lo)


def _host_prep(hidden_states, cos, sin, qkv_w, qkv_b, proj_w):
    cos_rep = np.tile(np.ascontiguousarray(cos.T), (HPC, 1))
    sin_rep = np.tile(np.ascontiguousarray(sin.T), (HPC, 1))
    cs = np.ascontiguousarray(
        np.concatenate([cos_rep, sin_rep], axis=0)).astype(BF)
    ident = np.eye(128, dtype=BF)

    xs = [None, None]
    for b in range(B):
        xbf = hidden_states[b].T.astype(BF).astype(np.float32)
        xh, xl = _split8(xbf)
        xs[b] = np.ascontiguousarray(np.concatenate([xh, xl], axis=0))

    in_maps = []
    for c in range(NCORES):
        b = c // CORES_PER_B
        h0 = (c % CORES_PER_B) * HPC
        heads = list(range(h0, h0 + HPC))
        ev = [h * D + 2 * j for h in heads for j in range(D // 2)]
        od = [h * D + 2 * j + 1 for h in heads for j in range(D // 2)]
        perm = ev + od
        vrows = [h * D + dd for h in heads for dd in range(D)]

        def wsplit(wmat):  # [C, RL] f32, pre-scaled x32 via bf16 values
            wbf = (wmat.astype(BF).astype(np.float32)) * 32.0
            return _split8(wbf)

        wqh, wql = wsplit(qkv_w[0 * H * D:1 * H * D][perm].T)
        wkh, wkl = wsplit(qkv_w[1 * H * D:2 * H * D][perm].T)
        wvh, wvl = wsplit(qkv_w[2 * H * D:3 * H * D][vrows].T)
        bq = qkv_b[0 * H * D:1 * H * D][perm]
        bk = qkv_b[1 * H * D:2 * H * D][perm]
        bqk = np.ascontiguousarray(np.stack(
            [bq[0:128], bq[128:256], bk[0:128], bk[128:256]],
            axis=1).astype(np.float32))
        in_maps.append(dict(
            x8=xs[b],
            wqkv=np.ascontiguousarray(np.concatenate(
                [wqh, wql, wkh, wkl, wvh, wvl], axis=1)),
            wproj_t=np.ascontiguousarray(proj_w[:, vrows].T).astype(BF),
            bqk=bqk,
            bv=np.ascontiguousarray(qkv_b[2 * H * D:3 * H * D][vrows]),
            cs=cs, ident=ident,
        ))
    return in_maps


def kernel(hidden_states, cos, sin, qkv_w, qkv_b, proj_w, proj_b):
    hidden_states = np.asarray(hidden_states, dtype=np.float32)
    cos = np.asarray(cos, dtype=np.float32)
    sin = np.asarray(sin, dtype=np.float32)
    qkv_w = np.asarray(qkv_w, dtype=np.float32)
    qkv_b = np.asarray(qkv_b, dtype=np.float32)
    proj_w = np.asarray(proj_w, dtype=np.float32)
    proj_b = np.asarray(proj_b, dtype=np.float32)

    if "nc" not in _compiled:
        _compiled["nc"] = _build()
    nc = _compiled["nc"]

    in_maps = _host_prep(hidden_states, cos, sin, qkv_w, qkv_b, proj_w)
    res = run_bass_kernel_spmd(nc, in_maps, core_ids=list(range(NCORES)))
    outs = [np.asarray(res.results[c]["out"], dtype=np.float32)
            for c in range(NCORES)]
    final = np.empty((B, T, C), np.float32)
    for b in range(B):
        acc = outs[b * CORES_PER_B].copy()
        for i in range(1, CORES_PER_B):
            acc += outs[b * CORES_PER_B + i]
        final[b] = acc + proj_b[None, :]
    return final
